# revision 28
# baseline (speedup 1.0000x reference)
"""Trainium2 Bass kernel for a masked tree-GRU step (nn_Encoder_Base).

Reference semantics (B=262144 rows, hidden H=128, d=H):
    s  = hard_sigmoid(x @ W[:, :3d] + h_tm1 @ U[:, :3d] + b[:3d])
    z, r1, r2 = s split
    h_cand = tanh((r1*x) @ W[:, 3d:] + (r2*h_tm1) @ U[:, 3d:] + b[3d:])
    h_ = z*h_tm1 + (1-z)*h_cand
    h  = where(has_value_tm1, h_, x); h = where(merge, h, h_tm1)
    has_value = merge | has_value_tm1        (merge = x_mask & prev_has_value)

Strategy: pure data-parallel over 8 NeuronCores (32768 rows/core).

Device kernel works entirely in transposed space (features on partitions,
rows on the free axis), 512 rows per iteration:
  - host uploads x.T and h_eff.T as bf16, where h_eff pre-substitutes x
    into h_tm1 on rows whose output is x (merge & !hv)  [input marshaling]
  - rows whose output is a pass-through (everything except merge & hv)
    get +BIG added to the z-gate pre-activation via a K=1 matmul, so
    z saturates to 1 and the GRU recurrence emits h_eff unchanged
  - gates: 3 accumulated psum banks, one fused Relu over all 1536 cols
    (hard_sigmoid upper clip is fused into downstream scalar_tensor_tensor
    as min(.,1)); candidate tanh on ACT
  - 5 vector ops: r1*x, r2*h, h-hc, z*(h-hc), +hc
  - output written transposed bf16; host transposes back and upcasts.
"""
import sys

sys.path.insert(0, "/opt/trn_rl_repo")

import numpy as np

N_CORES = 8
B_FULL = 262144
H = 128
D = H
SHARD = B_FULL // N_CORES  # 32768
R = 512                    # rows per iteration
BIGM = 50.0

_CACHE = {}


def _build_program(shard_rows, zero_b=True, io_dt="bf16"):
    import concourse.tile as tile
    from concourse import bacc, mybir
    from contextlib import ExitStack

    f32, u8 = mybir.dt.float32, mybir.dt.uint8
    c16 = mybir.dt.bfloat16 if io_dt == "bf16" else mybir.dt.float16
    Alu = mybir.AluOpType
    Act = mybir.ActivationFunctionType

    iters = shard_rows // R
    jc = shard_rows // H

    nc = bacc.Bacc("TRN2", target_bir_lowering=False, debug=False,
                   num_devices=N_CORES)
    xt_e = nc.dram_tensor("xt", [H, shard_rows], c16, kind="ExternalInput").ap()
    ht_e = nc.dram_tensor("ht", [H, shard_rows], c16, kind="ExternalInput").ap()
    w_e = nc.dram_tensor("w16", [H, 4 * D], c16, kind="ExternalInput").ap()
    u_e = nc.dram_tensor("u16", [H, 4 * D], c16, kind="ExternalInput").ap()
    on_e = nc.dram_tensor("ones1", [1, H], c16, kind="ExternalInput").ap()
    bg_e = nc.dram_tensor("biasg", [H, 4], f32, kind="ExternalInput").ap()
    mb_e = nc.dram_tensor("mbig", [1, shard_rows], c16,
                          kind="ExternalInput").ap()
    mg_e = nc.dram_tensor("mgc", [H, jc], f32, kind="ExternalInput").ap()
    hv_e = nc.dram_tensor("hvc", [H, jc], f32, kind="ExternalInput").ap()
    ho_e = nc.dram_tensor("hot", [H, shard_rows], c16,
                          kind="ExternalOutput").ap()
    vo_e = nc.dram_tensor("hvo", [shard_rows], u8, kind="ExternalOutput").ap()

    with tile.TileContext(nc) as tc, ExitStack() as ctx:
        consts = ctx.enter_context(tc.tile_pool(name="consts", bufs=1))
        w16 = consts.tile([H, 4 * D], c16)
        nc.sync.dma_start(w16[:], w_e[:])
        u16 = consts.tile([H, 4 * D], c16)
        nc.sync.dma_start(u16[:], u_e[:])
        ones1 = consts.tile([1, H], c16)
        nc.sync.dma_start(ones1[:], on_e[:])
        biasg = consts.tile([H, 4], f32)
        nc.sync.dma_start(biasg[:], bg_e[:])
        mbig = consts.tile([1, shard_rows], c16)
        nc.sync.dma_start(mbig[:], mb_e[:])
        mgc = consts.tile([H, jc], f32)
        nc.sync.dma_start(mgc[:], mg_e[:])
        hvc = consts.tile([H, jc], f32)
        nc.sync.dma_start(hvc[:], hv_e[:])

        # has_value = merge | hv_tm1  (0/1 floats -> max), then to uint8
        hvf = consts.tile([H, jc], f32)
        nc.vector.tensor_tensor(hvf[:], mgc[:], hvc[:], Alu.max)
        hvu = consts.tile([H, jc], u8)
        nc.vector.tensor_copy(hvu[:], hvf[:])
        nc.gpsimd.dma_start(vo_e.rearrange("(p j) -> p j", p=H), hvu[:])

        xin = ctx.enter_context(tc.tile_pool(name="xin", bufs=6))
        hin = ctx.enter_context(tc.tile_pool(name="hin", bufs=6))
        gat = ctx.enter_context(tc.tile_pool(name="gat", bufs=3))
        prd = ctx.enter_context(tc.tile_pool(name="prd", bufs=4))
        outp = ctx.enter_context(tc.tile_pool(name="outp", bufs=6))
        p_g = ctx.enter_context(tc.tile_pool(name="pg", bufs=2, space="PSUM"))
        p_c = ctx.enter_context(tc.tile_pool(name="pc", bufs=2, space="PSUM"))

        for i in range(iters):
            r0 = i * R
            xTt = xin.tile([H, R], c16, tag="xT")
            nc.sync.dma_start(xTt[:], xt_e[:, r0:r0 + R])
            hTt = hin.tile([H, R], c16, tag="hT")
            nc.sync.dma_start(hTt[:], ht_e[:, r0:r0 + R])
            xT = xTt[:]
            hT = hTt[:]

            # gates psum: [z | r1 | r2] in 3 consecutive banks
            pg = p_g.tile([H, 3 * R], f32, tag="pg")
            pz, pr1, pr2 = pg[:, 0:R], pg[:, R:2 * R], pg[:, 2 * R:3 * R]
            nc.tensor.matmul(pz, w16[:, 0:D], xT, start=True, stop=False)
            nc.tensor.matmul(pz, u16[:, 0:D], hT, start=False, stop=False)
            nc.tensor.matmul(pz, ones1[:], mbig[0:1, r0:r0 + R], start=False,
                             stop=True)
            nc.tensor.matmul(pr1, w16[:, D:2 * D], xT, start=True,
                             stop=False)
            nc.tensor.matmul(pr1, u16[:, D:2 * D], hT, start=False,
                             stop=True)
            nc.tensor.matmul(pr2, w16[:, 2 * D:3 * D], xT, start=True,
                             stop=False)
            nc.tensor.matmul(pr2, u16[:, 2 * D:3 * D], hT, start=False,
                             stop=True)

            rall = gat.tile([H, 3 * R], c16, tag="rall")
            if zero_b:
                # b == 0: all three gate biases are the same 0.5 column
                nc.scalar.activation(rall[:], pg[:], Act.Relu,
                                     bias=biasg[:, 0:1], scale=0.2)
            else:
                nc.scalar.activation(rall[:, 0:R], pz, Act.Relu,
                                     bias=biasg[:, 0:1], scale=0.2)
                nc.scalar.activation(rall[:, R:2 * R], pr1, Act.Relu,
                                     bias=biasg[:, 1:2], scale=0.2)
                nc.scalar.activation(rall[:, 2 * R:3 * R], pr2, Act.Relu,
                                     bias=biasg[:, 2:3], scale=0.2)

            r1x = prd.tile([H, R], c16, tag="r1x")
            nc.vector.scalar_tensor_tensor(r1x[:], rall[:, R:2 * R], 1.0,
                                           xT, Alu.min, Alu.mult)
            r2h = prd.tile([H, R], c16, tag="r2h")
            nc.vector.scalar_tensor_tensor(r2h[:], rall[:, 2 * R:3 * R], 1.0,
                                           hT, Alu.min, Alu.mult)

            pc = p_c.tile([H, R], f32, tag="pc")
            nc.tensor.matmul(pc[:], w16[:, 3 * D:4 * D], r1x[:], start=True,
                             stop=False)
            nc.tensor.matmul(pc[:], u16[:, 3 * D:4 * D], r2h[:], start=False,
                             stop=True)
            hc = gat.tile([H, R], c16, tag="hc")
            nc.scalar.activation(hc[:], pc[:], Act.Tanh, bias=biasg[:, 3:4],
                                 scale=1.0)

            t2 = prd.tile([H, R], c16, tag="t2")
            nc.gpsimd.tensor_tensor(t2[:], hT, hc[:], Alu.subtract)
            uu = prd.tile([H, R], c16, tag="uu")
            nc.vector.scalar_tensor_tensor(uu[:], rall[:, 0:R], 1.0, t2[:],
                                           Alu.min, Alu.mult)
            hout = outp.tile([H, R], c16, tag="hout")
            nc.vector.tensor_tensor(hout[:], uu[:], hc[:], Alu.add)

            nc.gpsimd.dma_start(ho_e[:, r0:r0 + R], hout[:])

    nc.compile()
    return nc


def _get_program(shard_rows=SHARD, zero_b=True, io_dt="bf16"):
    key = (shard_rows, zero_b, io_dt)
    if key not in _CACHE:
        _CACHE[key] = _build_program(shard_rows, zero_b, io_dt)
    return _CACHE[key]


def make_in_maps(x, h_tm1, W, U, b, x_mask, prev_has_value, has_value_tm1,
                 shard_rows=SHARD, n_cores=N_CORES, io_dt="bf16"):
    if io_dt == "bf16":
        import ml_dtypes
        cdt = ml_dtypes.bfloat16
    else:
        cdt = np.float16
    x = np.asarray(x, dtype=np.float32)
    h_tm1 = np.asarray(h_tm1, dtype=np.float32)
    W = np.asarray(W, dtype=np.float32)
    U = np.asarray(U, dtype=np.float32)
    b = np.asarray(b, dtype=np.float32)
    x_mask = np.asarray(x_mask)
    prev_has_value = np.asarray(prev_has_value)
    has_value_tm1 = np.asarray(has_value_tm1)

    w16 = np.ascontiguousarray(W, dtype=cdt)
    u16 = np.ascontiguousarray(U, dtype=cdt)
    ones1 = np.ones((1, H), cdt)
    biasg = np.zeros((H, 4), np.float32)
    biasg[:, 0] = 0.2 * b[0:D] + 0.5
    biasg[:, 1] = 0.2 * b[D:2 * D] + 0.5
    biasg[:, 2] = 0.2 * b[2 * D:3 * D] + 0.5
    biasg[:, 3] = b[3 * D:4 * D]

    jc = shard_rows // H
    in_maps = []
    for c in range(n_cores):
        sl = slice(c * shard_rows, (c + 1) * shard_rows)
        hv = has_value_tm1[sl] != 0
        merge = (x_mask[sl] * prev_has_value[sl]) != 0
        xpass = merge & ~hv           # rows whose output is x
        gru = merge & hv              # rows that really run the GRU
        # substitute x into h on x-pass rows; z-saturation then emits it
        h_eff = np.where(xpass[:, None], x[sl], h_tm1[sl])
        xt = np.ascontiguousarray(x[sl].astype(cdt).T)
        ht = np.ascontiguousarray(h_eff.astype(cdt).T)
        mbig = np.where(gru, cdt(0.0), cdt(BIGM)).reshape(1, shard_rows)
        mgc = merge.astype(np.float32).reshape(H, jc)
        hvc = hv.astype(np.float32).reshape(H, jc)
        in_maps.append({
            "xt": xt, "ht": ht, "w16": w16, "u16": u16, "ones1": ones1,
            "biasg": biasg, "mbig": mbig, "mgc": mgc, "hvc": hvc,
        })
    return in_maps


def kernel(x, h_tm1, W, U, b, x_mask, prev_has_value, has_value_tm1):
    from concourse.bass_utils import run_bass_kernel_spmd

    b = np.asarray(b, dtype=np.float32)
    zero_b = bool(np.all(b == 0.0))
    nc = _get_program(SHARD, zero_b=zero_b)
    in_maps = make_in_maps(x, h_tm1, W, U, b, x_mask, prev_has_value,
                           has_value_tm1)
    res = run_bass_kernel_spmd(nc, in_maps, list(range(N_CORES)))
    h = np.concatenate(
        [np.asarray(res.results[i]["hot"]).T.astype(np.float32)
         for i in range(N_CORES)], axis=0)
    hv = np.concatenate([res.results[i]["hvo"] for i in range(N_CORES)],
                        axis=0).astype(bool)
    return h, hv


# revision 30
# speedup vs baseline: 1.0075x; 1.0075x over previous
"""Trainium2 Bass kernel for a masked tree-GRU step (nn_Encoder_Base).

Reference semantics (B=262144 rows, hidden H=128, d=H):
    s  = hard_sigmoid(x @ W[:, :3d] + h_tm1 @ U[:, :3d] + b[:3d])
    z, r1, r2 = s split
    h_cand = tanh((r1*x) @ W[:, 3d:] + (r2*h_tm1) @ U[:, 3d:] + b[3d:])
    h_ = z*h_tm1 + (1-z)*h_cand
    h  = where(has_value_tm1, h_, x); h = where(merge, h, h_tm1)
    has_value = merge | has_value_tm1        (merge = x_mask & prev_has_value)

Strategy: pure data-parallel over 8 NeuronCores (32768 rows/core).

Device kernel works entirely in transposed space (features on partitions,
rows on the free axis), 512 rows per iteration:
  - host uploads x.T and h_eff.T as bf16, where h_eff pre-substitutes x
    into h_tm1 on rows whose output is x (merge & !hv)  [input marshaling]
  - rows whose output is a pass-through (everything except merge & hv)
    get +BIG added to the z-gate pre-activation via a K=1 matmul, so
    z saturates to 1 and the GRU recurrence emits h_eff unchanged
  - gates: 3 accumulated psum banks, one fused Relu over all 1536 cols
    (hard_sigmoid upper clip is fused into downstream scalar_tensor_tensor
    as min(.,1)); candidate tanh on ACT
  - 5 vector ops: r1*x, r2*h, h-hc, z*(h-hc), +hc
  - output written transposed bf16; host transposes back and upcasts.
"""
import sys

sys.path.insert(0, "/opt/trn_rl_repo")

import numpy as np

N_CORES = 8
B_FULL = 262144
H = 128
D = H
SHARD = B_FULL // N_CORES  # 32768
R = 512                    # rows per iteration
BIGM = 50.0

_CACHE = {}


def _build_program(shard_rows, zero_b=True, io_dt="bf16"):
    import concourse.tile as tile
    from concourse import bacc, mybir
    from contextlib import ExitStack

    f32, u8 = mybir.dt.float32, mybir.dt.uint8
    c16 = mybir.dt.bfloat16 if io_dt == "bf16" else mybir.dt.float16
    Alu = mybir.AluOpType
    Act = mybir.ActivationFunctionType

    iters = shard_rows // R
    jc = shard_rows // H

    nc = bacc.Bacc("TRN2", target_bir_lowering=False, debug=False,
                   num_devices=N_CORES)
    xt_e = nc.dram_tensor("xt", [H, shard_rows], c16, kind="ExternalInput").ap()
    ht_e = nc.dram_tensor("ht", [H, shard_rows], c16, kind="ExternalInput").ap()
    w_e = nc.dram_tensor("w16", [H, 4 * D], c16, kind="ExternalInput").ap()
    u_e = nc.dram_tensor("u16", [H, 4 * D], c16, kind="ExternalInput").ap()
    on_e = nc.dram_tensor("ones1", [1, H], c16, kind="ExternalInput").ap()
    bg_e = nc.dram_tensor("biasg", [H, 4], f32, kind="ExternalInput").ap()
    mb_e = nc.dram_tensor("mbig", [1, shard_rows], c16,
                          kind="ExternalInput").ap()
    mg_e = nc.dram_tensor("mgc", [H, jc], f32, kind="ExternalInput").ap()
    hv_e = nc.dram_tensor("hvc", [H, jc], f32, kind="ExternalInput").ap()
    ho_e = nc.dram_tensor("hot", [H, shard_rows], c16,
                          kind="ExternalOutput").ap()
    vo_e = nc.dram_tensor("hvo", [shard_rows], u8, kind="ExternalOutput").ap()

    with tile.TileContext(nc) as tc, ExitStack() as ctx:
        consts = ctx.enter_context(tc.tile_pool(name="consts", bufs=1))
        w16 = consts.tile([H, 4 * D], c16)
        nc.sync.dma_start(w16[:], w_e[:])
        u16 = consts.tile([H, 4 * D], c16)
        nc.sync.dma_start(u16[:], u_e[:])
        ones1 = consts.tile([1, H], c16)
        nc.sync.dma_start(ones1[:], on_e[:])
        biasg = consts.tile([H, 4], f32)
        nc.sync.dma_start(biasg[:], bg_e[:])
        mbig = consts.tile([1, shard_rows], c16)
        nc.sync.dma_start(mbig[:], mb_e[:])
        mgc = consts.tile([H, jc], f32)
        nc.sync.dma_start(mgc[:], mg_e[:])
        hvc = consts.tile([H, jc], f32)
        nc.sync.dma_start(hvc[:], hv_e[:])

        # has_value = merge | hv_tm1  (0/1 floats -> max), then to uint8
        hvf = consts.tile([H, jc], f32)
        nc.vector.tensor_tensor(hvf[:], mgc[:], hvc[:], Alu.max)
        hvu = consts.tile([H, jc], u8)
        nc.vector.tensor_copy(hvu[:], hvf[:])
        nc.gpsimd.dma_start(vo_e.rearrange("(p j) -> p j", p=H), hvu[:])

        xin = ctx.enter_context(tc.tile_pool(name="xin", bufs=6))
        hin = ctx.enter_context(tc.tile_pool(name="hin", bufs=6))
        gat = ctx.enter_context(tc.tile_pool(name="gat", bufs=3))
        prd = ctx.enter_context(tc.tile_pool(name="prd", bufs=4))
        outp = ctx.enter_context(tc.tile_pool(name="outp", bufs=6))
        p_g = ctx.enter_context(tc.tile_pool(name="pg", bufs=2, space="PSUM"))
        p_c = ctx.enter_context(tc.tile_pool(name="pc", bufs=2, space="PSUM"))

        for i in range(iters):
            r0 = i * R
            xTt = xin.tile([H, R], c16, tag="xT")
            nc.sync.dma_start(xTt[:], xt_e[:, r0:r0 + R])
            hTt = hin.tile([H, R], c16, tag="hT")
            nc.sync.dma_start(hTt[:], ht_e[:, r0:r0 + R])
            xT = xTt[:]
            hT = hTt[:]

            # gates psum: [z | r1 | r2] in 3 consecutive banks
            pg = p_g.tile([H, 3 * R], f32, tag="pg")
            pz, pr1, pr2 = pg[:, 0:R], pg[:, R:2 * R], pg[:, 2 * R:3 * R]
            nc.tensor.matmul(pz, w16[:, 0:D], xT, start=True, stop=False)
            nc.tensor.matmul(pz, u16[:, 0:D], hT, start=False, stop=False)
            nc.tensor.matmul(pz, ones1[:], mbig[0:1, r0:r0 + R], start=False,
                             stop=True)
            nc.tensor.matmul(pr1, w16[:, D:2 * D], xT, start=True,
                             stop=False)
            nc.tensor.matmul(pr1, u16[:, D:2 * D], hT, start=False,
                             stop=True)
            nc.tensor.matmul(pr2, w16[:, 2 * D:3 * D], xT, start=True,
                             stop=False)
            nc.tensor.matmul(pr2, u16[:, 2 * D:3 * D], hT, start=False,
                             stop=True)

            rall = gat.tile([H, 3 * R], c16, tag="rall")
            if zero_b:
                # b == 0: all three gate biases are the same 0.5 column
                nc.scalar.activation(rall[:], pg[:], Act.Relu,
                                     bias=biasg[:, 0:1], scale=0.2)
            else:
                nc.scalar.activation(rall[:, 0:R], pz, Act.Relu,
                                     bias=biasg[:, 0:1], scale=0.2)
                nc.scalar.activation(rall[:, R:2 * R], pr1, Act.Relu,
                                     bias=biasg[:, 1:2], scale=0.2)
                nc.scalar.activation(rall[:, 2 * R:3 * R], pr2, Act.Relu,
                                     bias=biasg[:, 2:3], scale=0.2)

            r1x = prd.tile([H, R], c16, tag="r1x")
            nc.vector.scalar_tensor_tensor(r1x[:], rall[:, R:2 * R], 1.0,
                                           xT, Alu.min, Alu.mult)
            r2h = prd.tile([H, R], c16, tag="r2h")
            nc.vector.scalar_tensor_tensor(r2h[:], rall[:, 2 * R:3 * R], 1.0,
                                           hT, Alu.min, Alu.mult)

            pc = p_c.tile([H, R], f32, tag="pc")
            nc.tensor.matmul(pc[:], w16[:, 3 * D:4 * D], r1x[:], start=True,
                             stop=False)
            nc.tensor.matmul(pc[:], u16[:, 3 * D:4 * D], r2h[:], start=False,
                             stop=True)
            hc = gat.tile([H, R], c16, tag="hc")
            nc.scalar.activation(hc[:], pc[:], Act.Tanh, bias=biasg[:, 3:4],
                                 scale=1.0)

            t2 = prd.tile([H, R], c16, tag="t2")
            nc.vector.tensor_tensor(t2[:], hT, hc[:], Alu.subtract)
            uu = prd.tile([H, R], c16, tag="uu")
            nc.vector.scalar_tensor_tensor(uu[:], rall[:, 0:R], 1.0, t2[:],
                                           Alu.min, Alu.mult)
            hout = outp.tile([H, R], c16, tag="hout")
            nc.gpsimd.tensor_tensor(hout[:], uu[:], hc[:], Alu.add)

            nc.gpsimd.dma_start(ho_e[:, r0:r0 + R], hout[:])

    nc.compile()
    return nc


def _get_program(shard_rows=SHARD, zero_b=True, io_dt="bf16"):
    key = (shard_rows, zero_b, io_dt)
    if key not in _CACHE:
        _CACHE[key] = _build_program(shard_rows, zero_b, io_dt)
    return _CACHE[key]


def make_in_maps(x, h_tm1, W, U, b, x_mask, prev_has_value, has_value_tm1,
                 shard_rows=SHARD, n_cores=N_CORES, io_dt="bf16"):
    if io_dt == "bf16":
        import ml_dtypes
        cdt = ml_dtypes.bfloat16
    else:
        cdt = np.float16
    x = np.asarray(x, dtype=np.float32)
    h_tm1 = np.asarray(h_tm1, dtype=np.float32)
    W = np.asarray(W, dtype=np.float32)
    U = np.asarray(U, dtype=np.float32)
    b = np.asarray(b, dtype=np.float32)
    x_mask = np.asarray(x_mask)
    prev_has_value = np.asarray(prev_has_value)
    has_value_tm1 = np.asarray(has_value_tm1)

    w16 = np.ascontiguousarray(W, dtype=cdt)
    u16 = np.ascontiguousarray(U, dtype=cdt)
    ones1 = np.ones((1, H), cdt)
    biasg = np.zeros((H, 4), np.float32)
    biasg[:, 0] = 0.2 * b[0:D] + 0.5
    biasg[:, 1] = 0.2 * b[D:2 * D] + 0.5
    biasg[:, 2] = 0.2 * b[2 * D:3 * D] + 0.5
    biasg[:, 3] = b[3 * D:4 * D]

    jc = shard_rows // H
    in_maps = []
    for c in range(n_cores):
        sl = slice(c * shard_rows, (c + 1) * shard_rows)
        hv = has_value_tm1[sl] != 0
        merge = (x_mask[sl] * prev_has_value[sl]) != 0
        xpass = merge & ~hv           # rows whose output is x
        gru = merge & hv              # rows that really run the GRU
        # substitute x into h on x-pass rows; z-saturation then emits it
        h_eff = np.where(xpass[:, None], x[sl], h_tm1[sl])
        xt = np.ascontiguousarray(x[sl].astype(cdt).T)
        ht = np.ascontiguousarray(h_eff.astype(cdt).T)
        mbig = np.where(gru, cdt(0.0), cdt(BIGM)).reshape(1, shard_rows)
        mgc = merge.astype(np.float32).reshape(H, jc)
        hvc = hv.astype(np.float32).reshape(H, jc)
        in_maps.append({
            "xt": xt, "ht": ht, "w16": w16, "u16": u16, "ones1": ones1,
            "biasg": biasg, "mbig": mbig, "mgc": mgc, "hvc": hvc,
        })
    return in_maps


def kernel(x, h_tm1, W, U, b, x_mask, prev_has_value, has_value_tm1):
    from concourse.bass_utils import run_bass_kernel_spmd

    b = np.asarray(b, dtype=np.float32)
    zero_b = bool(np.all(b == 0.0))
    nc = _get_program(SHARD, zero_b=zero_b)
    in_maps = make_in_maps(x, h_tm1, W, U, b, x_mask, prev_has_value,
                           has_value_tm1)
    res = run_bass_kernel_spmd(nc, in_maps, list(range(N_CORES)))
    h = np.concatenate(
        [np.asarray(res.results[i]["hot"]).T.astype(np.float32)
         for i in range(N_CORES)], axis=0)
    hv = np.concatenate([res.results[i]["hvo"] for i in range(N_CORES)],
                        axis=0).astype(bool)
    return h, hv


# revision 33
# speedup vs baseline: 1.2537x; 1.2443x over previous
"""Trainium2 Bass kernel for a masked tree-GRU step (nn_Encoder_Base).

Reference semantics (B=262144 rows, hidden H=128, d=H):
    s  = hard_sigmoid(x @ W[:, :3d] + h_tm1 @ U[:, :3d] + b[:3d])
    z, r1, r2 = s split
    h_cand = tanh((r1*x) @ W[:, 3d:] + (r2*h_tm1) @ U[:, 3d:] + b[3d:])
    h_ = z*h_tm1 + (1-z)*h_cand
    h  = where(has_value_tm1, h_, x); h = where(merge, h, h_tm1)
    has_value = merge | has_value_tm1        (merge = x_mask & prev_has_value)

Strategy: pure data-parallel over 8 NeuronCores (32768 rows/core).

Device kernel works entirely in transposed space (features on partitions,
rows on the free axis), 512 rows per iteration:
  - host uploads x.T and h_eff.T as bf16, where h_eff pre-substitutes x
    into h_tm1 on rows whose output is x (merge & !hv)  [input marshaling]
  - rows whose output is a pass-through (everything except merge & hv)
    get +BIG added to the z-gate pre-activation via a K=1 matmul, so
    z saturates to 1 and the GRU recurrence emits h_eff unchanged
  - gates: 3 accumulated psum banks, one fused Relu over all 1536 cols
    (hard_sigmoid upper clip is fused into downstream scalar_tensor_tensor
    as min(.,1)); candidate tanh on ACT
  - 5 vector ops: r1*x, r2*h, h-hc, z*(h-hc), +hc
  - output written transposed bf16; host transposes back and upcasts.
"""
import sys

sys.path.insert(0, "/opt/trn_rl_repo")

import numpy as np

N_CORES = 8
B_FULL = 262144
H = 128
D = H
SHARD = B_FULL // N_CORES  # 32768
R = 512                    # rows per iteration
BIGM = 50.0

_CACHE = {}


def _build_program(shard_rows, zero_b=True, io_dt="bf16"):
    import concourse.tile as tile
    from concourse import bacc, mybir
    from contextlib import ExitStack

    f32, u8 = mybir.dt.float32, mybir.dt.uint8
    c16 = mybir.dt.bfloat16 if io_dt == "bf16" else mybir.dt.float16
    Alu = mybir.AluOpType
    Act = mybir.ActivationFunctionType

    iters = shard_rows // R
    jc = shard_rows // H

    nc = bacc.Bacc("TRN2", target_bir_lowering=False, debug=False,
                   num_devices=N_CORES)
    xt_e = nc.dram_tensor("xt", [H, shard_rows], c16, kind="ExternalInput").ap()
    ht_e = nc.dram_tensor("ht", [H, shard_rows], c16, kind="ExternalInput").ap()
    w_e = nc.dram_tensor("w16", [H, 4 * D], c16, kind="ExternalInput").ap()
    u_e = nc.dram_tensor("u16", [H, 4 * D], c16, kind="ExternalInput").ap()
    on_e = nc.dram_tensor("ones1", [1, H], c16, kind="ExternalInput").ap()
    bg_e = nc.dram_tensor("biasg", [H, 4], f32, kind="ExternalInput").ap()
    mb_e = nc.dram_tensor("mbig", [1, shard_rows], c16,
                          kind="ExternalInput").ap()
    mg_e = nc.dram_tensor("mgc", [H, jc], f32, kind="ExternalInput").ap()
    hv_e = nc.dram_tensor("hvc", [H, jc], f32, kind="ExternalInput").ap()
    ho_e = nc.dram_tensor("hot", [H, shard_rows], c16,
                          kind="ExternalOutput").ap()
    vo_e = nc.dram_tensor("hvo", [shard_rows], u8, kind="ExternalOutput").ap()

    with tile.TileContext(nc) as tc, ExitStack() as ctx:
        consts = ctx.enter_context(tc.tile_pool(name="consts", bufs=1))
        w16 = consts.tile([H, 4 * D], c16)
        nc.sync.dma_start(w16[:], w_e[:])
        u16 = consts.tile([H, 4 * D], c16)
        nc.sync.dma_start(u16[:], u_e[:])
        ones1 = consts.tile([1, H], c16)
        nc.sync.dma_start(ones1[:], on_e[:])
        biasg = consts.tile([H, 4], f32)
        nc.sync.dma_start(biasg[:], bg_e[:])
        mbig = consts.tile([1, shard_rows], c16)
        nc.sync.dma_start(mbig[:], mb_e[:])
        mgc = consts.tile([H, jc], f32)
        nc.sync.dma_start(mgc[:], mg_e[:])
        hvc = consts.tile([H, jc], f32)
        nc.sync.dma_start(hvc[:], hv_e[:])

        # has_value = merge | hv_tm1  (0/1 floats -> max), then to uint8
        hvf = consts.tile([H, jc], f32)
        nc.vector.tensor_tensor(hvf[:], mgc[:], hvc[:], Alu.max)
        hvu = consts.tile([H, jc], u8)
        nc.vector.tensor_copy(hvu[:], hvf[:])
        nc.gpsimd.dma_start(vo_e.rearrange("(p j) -> p j", p=H), hvu[:])

        xin = ctx.enter_context(tc.tile_pool(name="xin", bufs=6))
        hin = ctx.enter_context(tc.tile_pool(name="hin", bufs=6))
        gat = ctx.enter_context(tc.tile_pool(name="gat", bufs=3))
        prd = ctx.enter_context(tc.tile_pool(name="prd", bufs=4))
        outp = ctx.enter_context(tc.tile_pool(name="outp", bufs=6))
        p_g = ctx.enter_context(tc.tile_pool(name="pg", bufs=2, space="PSUM"))
        p_c = ctx.enter_context(tc.tile_pool(name="pc", bufs=2, space="PSUM"))

        for i in range(iters):
            r0 = i * R
            xTt = xin.tile([H, R], c16, tag="xT")
            nc.sync.dma_start(xTt[:], xt_e[:, r0:r0 + R])
            hTt = hin.tile([H, R], c16, tag="hT")
            nc.sync.dma_start(hTt[:], ht_e[:, r0:r0 + R])
            xT = xTt[:]
            hT = hTt[:]

            # gates psum: [z | r1 | r2] in 3 consecutive banks
            pg = p_g.tile([H, 3 * R], f32, tag="pg")
            pz, pr1, pr2 = pg[:, 0:R], pg[:, R:2 * R], pg[:, 2 * R:3 * R]
            nc.tensor.matmul(pz, w16[:, 0:D], xT, start=True, stop=False)
            nc.tensor.matmul(pz, u16[:, 0:D], hT, start=False, stop=False)
            nc.tensor.matmul(pz, ones1[:], mbig[0:1, r0:r0 + R], start=False,
                             stop=True)
            nc.tensor.matmul(pr1, w16[:, D:2 * D], xT, start=True,
                             stop=False)
            nc.tensor.matmul(pr1, u16[:, D:2 * D], hT, start=False,
                             stop=True)
            nc.tensor.matmul(pr2, w16[:, 2 * D:3 * D], xT, start=True,
                             stop=False)
            nc.tensor.matmul(pr2, u16[:, 2 * D:3 * D], hT, start=False,
                             stop=True)

            rall = gat.tile([H, 3 * R], c16, tag="rall")
            if zero_b:
                # b == 0: all three gate biases are the same 0.5 column
                nc.scalar.activation(rall[:], pg[:], Act.Relu,
                                     bias=biasg[:, 0:1], scale=0.2)
            else:
                nc.scalar.activation(rall[:, 0:R], pz, Act.Relu,
                                     bias=biasg[:, 0:1], scale=0.2)
                nc.scalar.activation(rall[:, R:2 * R], pr1, Act.Relu,
                                     bias=biasg[:, 1:2], scale=0.2)
                nc.scalar.activation(rall[:, 2 * R:3 * R], pr2, Act.Relu,
                                     bias=biasg[:, 2:3], scale=0.2)

            # hard_sigmoid upper clip once for all three gates (single-src
            # tensor_scalar runs in a fast DVE mode; the downstream muls
            # then run as plain tensor_tensor instead of 3 slower STTs)
            rclip = gat.tile([H, 3 * R], c16, tag="rclip")
            nc.vector.tensor_scalar_min(rclip[:], rall[:], 1.0)

            r1x = prd.tile([H, R], c16, tag="r1x")
            nc.vector.tensor_tensor(r1x[:], rclip[:, R:2 * R], xT, Alu.mult)
            r2h = prd.tile([H, R], c16, tag="r2h")
            nc.vector.tensor_tensor(r2h[:], rclip[:, 2 * R:3 * R], hT,
                                    Alu.mult)

            pc = p_c.tile([H, R], f32, tag="pc")
            nc.tensor.matmul(pc[:], w16[:, 3 * D:4 * D], r1x[:], start=True,
                             stop=False)
            nc.tensor.matmul(pc[:], u16[:, 3 * D:4 * D], r2h[:], start=False,
                             stop=True)
            hc = gat.tile([H, R], c16, tag="hc")
            nc.scalar.activation(hc[:], pc[:], Act.Tanh, bias=biasg[:, 3:4],
                                 scale=1.0)

            t2 = prd.tile([H, R], c16, tag="t2")
            nc.vector.tensor_tensor(t2[:], hT, hc[:], Alu.subtract)
            uu = prd.tile([H, R], c16, tag="uu")
            nc.vector.tensor_tensor(uu[:], rclip[:, 0:R], t2[:], Alu.mult)
            hout = outp.tile([H, R], c16, tag="hout")
            nc.vector.tensor_tensor(hout[:], uu[:], hc[:], Alu.add)

            nc.gpsimd.dma_start(ho_e[:, r0:r0 + R], hout[:])

    nc.compile()
    return nc


def _get_program(shard_rows=SHARD, zero_b=True, io_dt="bf16"):
    key = (shard_rows, zero_b, io_dt)
    if key not in _CACHE:
        _CACHE[key] = _build_program(shard_rows, zero_b, io_dt)
    return _CACHE[key]


def make_in_maps(x, h_tm1, W, U, b, x_mask, prev_has_value, has_value_tm1,
                 shard_rows=SHARD, n_cores=N_CORES, io_dt="bf16"):
    if io_dt == "bf16":
        import ml_dtypes
        cdt = ml_dtypes.bfloat16
    else:
        cdt = np.float16
    x = np.asarray(x, dtype=np.float32)
    h_tm1 = np.asarray(h_tm1, dtype=np.float32)
    W = np.asarray(W, dtype=np.float32)
    U = np.asarray(U, dtype=np.float32)
    b = np.asarray(b, dtype=np.float32)
    x_mask = np.asarray(x_mask)
    prev_has_value = np.asarray(prev_has_value)
    has_value_tm1 = np.asarray(has_value_tm1)

    w16 = np.ascontiguousarray(W, dtype=cdt)
    u16 = np.ascontiguousarray(U, dtype=cdt)
    ones1 = np.ones((1, H), cdt)
    biasg = np.zeros((H, 4), np.float32)
    biasg[:, 0] = 0.2 * b[0:D] + 0.5
    biasg[:, 1] = 0.2 * b[D:2 * D] + 0.5
    biasg[:, 2] = 0.2 * b[2 * D:3 * D] + 0.5
    biasg[:, 3] = b[3 * D:4 * D]

    jc = shard_rows // H
    in_maps = []
    for c in range(n_cores):
        sl = slice(c * shard_rows, (c + 1) * shard_rows)
        hv = has_value_tm1[sl] != 0
        merge = (x_mask[sl] * prev_has_value[sl]) != 0
        xpass = merge & ~hv           # rows whose output is x
        gru = merge & hv              # rows that really run the GRU
        # substitute x into h on x-pass rows; z-saturation then emits it
        h_eff = np.where(xpass[:, None], x[sl], h_tm1[sl])
        xt = np.ascontiguousarray(x[sl].astype(cdt).T)
        ht = np.ascontiguousarray(h_eff.astype(cdt).T)
        mbig = np.where(gru, cdt(0.0), cdt(BIGM)).reshape(1, shard_rows)
        mgc = merge.astype(np.float32).reshape(H, jc)
        hvc = hv.astype(np.float32).reshape(H, jc)
        in_maps.append({
            "xt": xt, "ht": ht, "w16": w16, "u16": u16, "ones1": ones1,
            "biasg": biasg, "mbig": mbig, "mgc": mgc, "hvc": hvc,
        })
    return in_maps


def kernel(x, h_tm1, W, U, b, x_mask, prev_has_value, has_value_tm1):
    from concourse.bass_utils import run_bass_kernel_spmd

    b = np.asarray(b, dtype=np.float32)
    zero_b = bool(np.all(b == 0.0))
    nc = _get_program(SHARD, zero_b=zero_b)
    in_maps = make_in_maps(x, h_tm1, W, U, b, x_mask, prev_has_value,
                           has_value_tm1)
    res = run_bass_kernel_spmd(nc, in_maps, list(range(N_CORES)))
    h = np.concatenate(
        [np.asarray(res.results[i]["hot"]).T.astype(np.float32)
         for i in range(N_CORES)], axis=0)
    hv = np.concatenate([res.results[i]["hvo"] for i in range(N_CORES)],
                        axis=0).astype(bool)
    return h, hv


# revision 34
# speedup vs baseline: 1.3393x; 1.0683x over previous
"""Trainium2 Bass kernel for a masked tree-GRU step (nn_Encoder_Base).

Reference semantics (B=262144 rows, hidden H=128, d=H):
    s  = hard_sigmoid(x @ W[:, :3d] + h_tm1 @ U[:, :3d] + b[:3d])
    z, r1, r2 = s split
    h_cand = tanh((r1*x) @ W[:, 3d:] + (r2*h_tm1) @ U[:, 3d:] + b[3d:])
    h_ = z*h_tm1 + (1-z)*h_cand
    h  = where(has_value_tm1, h_, x); h = where(merge, h, h_tm1)
    has_value = merge | has_value_tm1        (merge = x_mask & prev_has_value)

Strategy: pure data-parallel over 8 NeuronCores (32768 rows/core).

Device kernel works entirely in transposed space (features on partitions,
rows on the free axis), 512 rows per iteration:
  - host uploads x.T and h_eff.T as bf16, where h_eff pre-substitutes x
    into h_tm1 on rows whose output is x (merge & !hv)  [input marshaling]
  - rows whose output is a pass-through (everything except merge & hv)
    get +BIG added to the z-gate pre-activation via a K=1 matmul, so
    z saturates to 1 and the GRU recurrence emits h_eff unchanged
  - gates: 3 accumulated psum banks, one fused Relu over all 1536 cols
    (hard_sigmoid upper clip is fused into downstream scalar_tensor_tensor
    as min(.,1)); candidate tanh on ACT
  - 5 vector ops: r1*x, r2*h, h-hc, z*(h-hc), +hc
  - output written transposed bf16; host transposes back and upcasts.
"""
import sys

sys.path.insert(0, "/opt/trn_rl_repo")

import numpy as np

N_CORES = 8
B_FULL = 262144
H = 128
D = H
SHARD = B_FULL // N_CORES  # 32768
R = 512                    # rows per iteration
BIGM = 50.0

_CACHE = {}


def _build_program(shard_rows, zero_b=True, io_dt="bf16"):
    import concourse.tile as tile
    from concourse import bacc, mybir
    from contextlib import ExitStack

    f32, u8 = mybir.dt.float32, mybir.dt.uint8
    c16 = mybir.dt.bfloat16 if io_dt == "bf16" else mybir.dt.float16
    Alu = mybir.AluOpType
    Act = mybir.ActivationFunctionType

    iters = shard_rows // R
    jc = shard_rows // H

    nc = bacc.Bacc("TRN2", target_bir_lowering=False, debug=False,
                   num_devices=N_CORES)
    xt_e = nc.dram_tensor("xt", [H, shard_rows], c16, kind="ExternalInput").ap()
    ht_e = nc.dram_tensor("ht", [H, shard_rows], c16, kind="ExternalInput").ap()
    w_e = nc.dram_tensor("w16", [H, 4 * D], c16, kind="ExternalInput").ap()
    u_e = nc.dram_tensor("u16", [H, 4 * D], c16, kind="ExternalInput").ap()
    on_e = nc.dram_tensor("ones1", [1, H], c16, kind="ExternalInput").ap()
    bg_e = nc.dram_tensor("biasg", [H, 4], f32, kind="ExternalInput").ap()
    mb_e = nc.dram_tensor("mbig", [1, shard_rows], c16,
                          kind="ExternalInput").ap()
    mg_e = nc.dram_tensor("mgc", [H, jc], f32, kind="ExternalInput").ap()
    hv_e = nc.dram_tensor("hvc", [H, jc], f32, kind="ExternalInput").ap()
    ho_e = nc.dram_tensor("hot", [H, shard_rows], c16,
                          kind="ExternalOutput").ap()
    vo_e = nc.dram_tensor("hvo", [shard_rows], u8, kind="ExternalOutput").ap()

    with tile.TileContext(nc) as tc, ExitStack() as ctx:
        consts = ctx.enter_context(tc.tile_pool(name="consts", bufs=1))
        w16 = consts.tile([H, 4 * D], c16)
        nc.sync.dma_start(w16[:], w_e[:])
        u16 = consts.tile([H, 4 * D], c16)
        nc.sync.dma_start(u16[:], u_e[:])
        ones1 = consts.tile([1, H], c16)
        nc.sync.dma_start(ones1[:], on_e[:])
        biasg = consts.tile([H, 4], f32)
        nc.sync.dma_start(biasg[:], bg_e[:])
        mbig = consts.tile([1, shard_rows], c16)
        nc.sync.dma_start(mbig[:], mb_e[:])
        mgc = consts.tile([H, jc], f32)
        nc.sync.dma_start(mgc[:], mg_e[:])
        hvc = consts.tile([H, jc], f32)
        nc.sync.dma_start(hvc[:], hv_e[:])

        # has_value = merge | hv_tm1  (0/1 floats -> max), then to uint8
        hvf = consts.tile([H, jc], f32)
        nc.vector.tensor_tensor(hvf[:], mgc[:], hvc[:], Alu.max)
        hvu = consts.tile([H, jc], u8)
        nc.vector.tensor_copy(hvu[:], hvf[:])
        nc.gpsimd.dma_start(vo_e.rearrange("(p j) -> p j", p=H), hvu[:])

        xin = ctx.enter_context(tc.tile_pool(name="xin", bufs=8))
        hin = ctx.enter_context(tc.tile_pool(name="hin", bufs=8))
        gat = ctx.enter_context(tc.tile_pool(name="gat", bufs=4))
        prd = ctx.enter_context(tc.tile_pool(name="prd", bufs=6))
        outp = ctx.enter_context(tc.tile_pool(name="outp", bufs=8))
        p_g = ctx.enter_context(tc.tile_pool(name="pg", bufs=2, space="PSUM"))
        p_c = ctx.enter_context(tc.tile_pool(name="pc", bufs=2, space="PSUM"))

        for i in range(iters):
            r0 = i * R
            xTt = xin.tile([H, R], c16, tag="xT")
            nc.sync.dma_start(xTt[:], xt_e[:, r0:r0 + R])
            hTt = hin.tile([H, R], c16, tag="hT")
            nc.sync.dma_start(hTt[:], ht_e[:, r0:r0 + R])
            xT = xTt[:]
            hT = hTt[:]

            # gates psum: [z | r1 | r2] in 3 consecutive banks
            pg = p_g.tile([H, 3 * R], f32, tag="pg")
            pz, pr1, pr2 = pg[:, 0:R], pg[:, R:2 * R], pg[:, 2 * R:3 * R]
            nc.tensor.matmul(pz, w16[:, 0:D], xT, start=True, stop=False)
            nc.tensor.matmul(pz, u16[:, 0:D], hT, start=False, stop=False)
            nc.tensor.matmul(pz, ones1[:], mbig[0:1, r0:r0 + R], start=False,
                             stop=True)
            nc.tensor.matmul(pr1, w16[:, D:2 * D], xT, start=True,
                             stop=False)
            nc.tensor.matmul(pr1, u16[:, D:2 * D], hT, start=False,
                             stop=True)
            nc.tensor.matmul(pr2, w16[:, 2 * D:3 * D], xT, start=True,
                             stop=False)
            nc.tensor.matmul(pr2, u16[:, 2 * D:3 * D], hT, start=False,
                             stop=True)

            rall = gat.tile([H, 3 * R], c16, tag="rall")
            if zero_b:
                # b == 0: all three gate biases are the same 0.5 column
                nc.scalar.activation(rall[:], pg[:], Act.Relu,
                                     bias=biasg[:, 0:1], scale=0.2)
            else:
                nc.scalar.activation(rall[:, 0:R], pz, Act.Relu,
                                     bias=biasg[:, 0:1], scale=0.2)
                nc.scalar.activation(rall[:, R:2 * R], pr1, Act.Relu,
                                     bias=biasg[:, 1:2], scale=0.2)
                nc.scalar.activation(rall[:, 2 * R:3 * R], pr2, Act.Relu,
                                     bias=biasg[:, 2:3], scale=0.2)

            # hard_sigmoid upper clip once for all three gates (single-src
            # tensor_scalar runs in a fast DVE mode; the downstream muls
            # then run as plain tensor_tensor instead of 3 slower STTs)
            rclip = gat.tile([H, 3 * R], c16, tag="rclip")
            nc.vector.tensor_scalar_min(rclip[:], rall[:], 1.0)

            r1x = prd.tile([H, R], c16, tag="r1x")
            nc.vector.tensor_tensor(r1x[:], rclip[:, R:2 * R], xT, Alu.mult)
            r2h = prd.tile([H, R], c16, tag="r2h")
            nc.vector.tensor_tensor(r2h[:], rclip[:, 2 * R:3 * R], hT,
                                    Alu.mult)

            pc = p_c.tile([H, R], f32, tag="pc")
            nc.tensor.matmul(pc[:], w16[:, 3 * D:4 * D], r1x[:], start=True,
                             stop=False)
            nc.tensor.matmul(pc[:], u16[:, 3 * D:4 * D], r2h[:], start=False,
                             stop=True)
            hc = gat.tile([H, R], c16, tag="hc")
            nc.scalar.activation(hc[:], pc[:], Act.Tanh, bias=biasg[:, 3:4],
                                 scale=1.0)

            t2 = prd.tile([H, R], c16, tag="t2")
            nc.vector.tensor_tensor(t2[:], hT, hc[:], Alu.subtract)
            uu = prd.tile([H, R], c16, tag="uu")
            nc.vector.tensor_tensor(uu[:], rclip[:, 0:R], t2[:], Alu.mult)
            hout = outp.tile([H, R], c16, tag="hout")
            nc.vector.tensor_tensor(hout[:], uu[:], hc[:], Alu.add)

            nc.gpsimd.dma_start(ho_e[:, r0:r0 + R], hout[:])

    nc.compile()
    return nc


def _get_program(shard_rows=SHARD, zero_b=True, io_dt="bf16"):
    key = (shard_rows, zero_b, io_dt)
    if key not in _CACHE:
        _CACHE[key] = _build_program(shard_rows, zero_b, io_dt)
    return _CACHE[key]


def make_in_maps(x, h_tm1, W, U, b, x_mask, prev_has_value, has_value_tm1,
                 shard_rows=SHARD, n_cores=N_CORES, io_dt="bf16"):
    if io_dt == "bf16":
        import ml_dtypes
        cdt = ml_dtypes.bfloat16
    else:
        cdt = np.float16
    x = np.asarray(x, dtype=np.float32)
    h_tm1 = np.asarray(h_tm1, dtype=np.float32)
    W = np.asarray(W, dtype=np.float32)
    U = np.asarray(U, dtype=np.float32)
    b = np.asarray(b, dtype=np.float32)
    x_mask = np.asarray(x_mask)
    prev_has_value = np.asarray(prev_has_value)
    has_value_tm1 = np.asarray(has_value_tm1)

    w16 = np.ascontiguousarray(W, dtype=cdt)
    u16 = np.ascontiguousarray(U, dtype=cdt)
    ones1 = np.ones((1, H), cdt)
    biasg = np.zeros((H, 4), np.float32)
    biasg[:, 0] = 0.2 * b[0:D] + 0.5
    biasg[:, 1] = 0.2 * b[D:2 * D] + 0.5
    biasg[:, 2] = 0.2 * b[2 * D:3 * D] + 0.5
    biasg[:, 3] = b[3 * D:4 * D]

    jc = shard_rows // H
    in_maps = []
    for c in range(n_cores):
        sl = slice(c * shard_rows, (c + 1) * shard_rows)
        hv = has_value_tm1[sl] != 0
        merge = (x_mask[sl] * prev_has_value[sl]) != 0
        xpass = merge & ~hv           # rows whose output is x
        gru = merge & hv              # rows that really run the GRU
        # substitute x into h on x-pass rows; z-saturation then emits it
        h_eff = np.where(xpass[:, None], x[sl], h_tm1[sl])
        xt = np.ascontiguousarray(x[sl].astype(cdt).T)
        ht = np.ascontiguousarray(h_eff.astype(cdt).T)
        mbig = np.where(gru, cdt(0.0), cdt(BIGM)).reshape(1, shard_rows)
        mgc = merge.astype(np.float32).reshape(H, jc)
        hvc = hv.astype(np.float32).reshape(H, jc)
        in_maps.append({
            "xt": xt, "ht": ht, "w16": w16, "u16": u16, "ones1": ones1,
            "biasg": biasg, "mbig": mbig, "mgc": mgc, "hvc": hvc,
        })
    return in_maps


def kernel(x, h_tm1, W, U, b, x_mask, prev_has_value, has_value_tm1):
    from concourse.bass_utils import run_bass_kernel_spmd

    b = np.asarray(b, dtype=np.float32)
    zero_b = bool(np.all(b == 0.0))
    nc = _get_program(SHARD, zero_b=zero_b)
    in_maps = make_in_maps(x, h_tm1, W, U, b, x_mask, prev_has_value,
                           has_value_tm1)
    res = run_bass_kernel_spmd(nc, in_maps, list(range(N_CORES)))
    h = np.concatenate(
        [np.asarray(res.results[i]["hot"]).T.astype(np.float32)
         for i in range(N_CORES)], axis=0)
    hv = np.concatenate([res.results[i]["hvo"] for i in range(N_CORES)],
                        axis=0).astype(bool)
    return h, hv


# revision 36
# speedup vs baseline: 1.3442x; 1.0037x over previous
"""Trainium2 Bass kernel for a masked tree-GRU step (nn_Encoder_Base).

Reference semantics (B=262144 rows, hidden H=128, d=H):
    s  = hard_sigmoid(x @ W[:, :3d] + h_tm1 @ U[:, :3d] + b[:3d])
    z, r1, r2 = s split
    h_cand = tanh((r1*x) @ W[:, 3d:] + (r2*h_tm1) @ U[:, 3d:] + b[3d:])
    h_ = z*h_tm1 + (1-z)*h_cand
    h  = where(has_value_tm1, h_, x); h = where(merge, h, h_tm1)
    has_value = merge | has_value_tm1        (merge = x_mask & prev_has_value)

Strategy: pure data-parallel over 8 NeuronCores (32768 rows/core).

Device kernel works entirely in transposed space (features on partitions,
rows on the free axis), 512 rows per iteration:
  - host uploads x.T and h_eff.T as bf16, where h_eff pre-substitutes x
    into h_tm1 on rows whose output is x (merge & !hv)  [input marshaling]
  - rows whose output is a pass-through (everything except merge & hv)
    get +BIG added to the z-gate pre-activation via a K=1 matmul, so
    z saturates to 1 and the GRU recurrence emits h_eff unchanged
  - gates: 3 accumulated psum banks, one fused Relu over all 1536 cols,
    then one wide tensor_scalar min(.,1) (hard_sigmoid upper clip) so the
    gate products run as fast-mode tensor_tensor ops; candidate tanh on ACT
  - vector ops: clip, r1*x, r2*h, h-hc, z*(h-hc), +hc
  - output written transposed bf16; host transposes back and upcasts
  - deep SBUF tile-pool buffering (8/8/4/6/8) is load-bearing: it converts
    cross-iteration dependency stalls into overlap (228->173us measured).
"""
import sys

sys.path.insert(0, "/opt/trn_rl_repo")

import numpy as np

N_CORES = 8
B_FULL = 262144
H = 128
D = H
SHARD = B_FULL // N_CORES  # 32768
R = 512                    # rows per iteration
BIGM = 50.0

_CACHE = {}


def _build_program(shard_rows, zero_b=True, io_dt="bf16"):
    import concourse.tile as tile
    from concourse import bacc, mybir
    from contextlib import ExitStack

    f32, u8 = mybir.dt.float32, mybir.dt.uint8
    c16 = mybir.dt.bfloat16 if io_dt == "bf16" else mybir.dt.float16
    Alu = mybir.AluOpType
    Act = mybir.ActivationFunctionType

    iters = shard_rows // R
    jc = shard_rows // H

    nc = bacc.Bacc("TRN2", target_bir_lowering=False, debug=False,
                   num_devices=N_CORES)
    xt_e = nc.dram_tensor("xt", [H, shard_rows], c16, kind="ExternalInput").ap()
    ht_e = nc.dram_tensor("ht", [H, shard_rows], c16, kind="ExternalInput").ap()
    w_e = nc.dram_tensor("w16", [H, 4 * D], c16, kind="ExternalInput").ap()
    u_e = nc.dram_tensor("u16", [H, 4 * D], c16, kind="ExternalInput").ap()
    on_e = nc.dram_tensor("ones1", [1, H], c16, kind="ExternalInput").ap()
    bg_e = nc.dram_tensor("biasg", [H, 4], f32, kind="ExternalInput").ap()
    mb_e = nc.dram_tensor("mbig", [1, shard_rows], c16,
                          kind="ExternalInput").ap()
    mg_e = nc.dram_tensor("mgc", [H, jc], f32, kind="ExternalInput").ap()
    hv_e = nc.dram_tensor("hvc", [H, jc], f32, kind="ExternalInput").ap()
    ho_e = nc.dram_tensor("hot", [H, shard_rows], c16,
                          kind="ExternalOutput").ap()
    vo_e = nc.dram_tensor("hvo", [shard_rows], u8, kind="ExternalOutput").ap()

    with tile.TileContext(nc) as tc, ExitStack() as ctx:
        consts = ctx.enter_context(tc.tile_pool(name="consts", bufs=1))
        w16 = consts.tile([H, 4 * D], c16)
        nc.sync.dma_start(w16[:], w_e[:])
        u16 = consts.tile([H, 4 * D], c16)
        nc.sync.dma_start(u16[:], u_e[:])
        ones1 = consts.tile([1, H], c16)
        nc.sync.dma_start(ones1[:], on_e[:])
        biasg = consts.tile([H, 4], f32)
        nc.sync.dma_start(biasg[:], bg_e[:])
        mbig = consts.tile([1, shard_rows], c16)
        nc.sync.dma_start(mbig[:], mb_e[:])
        mgc = consts.tile([H, jc], f32)
        nc.sync.dma_start(mgc[:], mg_e[:])
        hvc = consts.tile([H, jc], f32)
        nc.sync.dma_start(hvc[:], hv_e[:])

        # has_value = merge | hv_tm1  (0/1 floats -> max), then to uint8
        hvf = consts.tile([H, jc], f32)
        nc.vector.tensor_tensor(hvf[:], mgc[:], hvc[:], Alu.max)
        hvu = consts.tile([H, jc], u8)
        nc.vector.tensor_copy(hvu[:], hvf[:])
        nc.gpsimd.dma_start(vo_e.rearrange("(p j) -> p j", p=H), hvu[:])

        xin = ctx.enter_context(tc.tile_pool(name="xin", bufs=10))
        hin = ctx.enter_context(tc.tile_pool(name="hin", bufs=10))
        gat = ctx.enter_context(tc.tile_pool(name="gat", bufs=5))
        prd = ctx.enter_context(tc.tile_pool(name="prd", bufs=7))
        outp = ctx.enter_context(tc.tile_pool(name="outp", bufs=10))
        p_g = ctx.enter_context(tc.tile_pool(name="pg", bufs=2, space="PSUM"))
        p_c = ctx.enter_context(tc.tile_pool(name="pc", bufs=2, space="PSUM"))

        for i in range(iters):
            r0 = i * R
            xTt = xin.tile([H, R], c16, tag="xT")
            nc.sync.dma_start(xTt[:], xt_e[:, r0:r0 + R])
            hTt = hin.tile([H, R], c16, tag="hT")
            nc.sync.dma_start(hTt[:], ht_e[:, r0:r0 + R])
            xT = xTt[:]
            hT = hTt[:]

            # gates psum: [z | r1 | r2] in 3 consecutive banks
            pg = p_g.tile([H, 3 * R], f32, tag="pg")
            pz, pr1, pr2 = pg[:, 0:R], pg[:, R:2 * R], pg[:, 2 * R:3 * R]
            nc.tensor.matmul(pz, w16[:, 0:D], xT, start=True, stop=False)
            nc.tensor.matmul(pz, u16[:, 0:D], hT, start=False, stop=False)
            nc.tensor.matmul(pz, ones1[:], mbig[0:1, r0:r0 + R], start=False,
                             stop=True)
            nc.tensor.matmul(pr1, w16[:, D:2 * D], xT, start=True,
                             stop=False)
            nc.tensor.matmul(pr1, u16[:, D:2 * D], hT, start=False,
                             stop=True)
            nc.tensor.matmul(pr2, w16[:, 2 * D:3 * D], xT, start=True,
                             stop=False)
            nc.tensor.matmul(pr2, u16[:, 2 * D:3 * D], hT, start=False,
                             stop=True)

            rall = gat.tile([H, 3 * R], c16, tag="rall")
            if zero_b:
                # b == 0: all three gate biases are the same 0.5 column
                nc.scalar.activation(rall[:], pg[:], Act.Relu,
                                     bias=biasg[:, 0:1], scale=0.2)
            else:
                nc.scalar.activation(rall[:, 0:R], pz, Act.Relu,
                                     bias=biasg[:, 0:1], scale=0.2)
                nc.scalar.activation(rall[:, R:2 * R], pr1, Act.Relu,
                                     bias=biasg[:, 1:2], scale=0.2)
                nc.scalar.activation(rall[:, 2 * R:3 * R], pr2, Act.Relu,
                                     bias=biasg[:, 2:3], scale=0.2)

            # hard_sigmoid upper clip once for all three gates (single-src
            # tensor_scalar runs in a fast DVE mode; the downstream muls
            # then run as plain tensor_tensor instead of 3 slower STTs)
            rclip = gat.tile([H, 3 * R], c16, tag="rclip")
            nc.vector.tensor_scalar_min(rclip[:], rall[:], 1.0)

            r1x = prd.tile([H, R], c16, tag="r1x")
            nc.vector.tensor_tensor(r1x[:], rclip[:, R:2 * R], xT, Alu.mult)
            r2h = prd.tile([H, R], c16, tag="r2h")
            nc.vector.tensor_tensor(r2h[:], rclip[:, 2 * R:3 * R], hT,
                                    Alu.mult)

            pc = p_c.tile([H, R], f32, tag="pc")
            nc.tensor.matmul(pc[:], w16[:, 3 * D:4 * D], r1x[:], start=True,
                             stop=False)
            nc.tensor.matmul(pc[:], u16[:, 3 * D:4 * D], r2h[:], start=False,
                             stop=True)
            hc = gat.tile([H, R], c16, tag="hc")
            nc.scalar.activation(hc[:], pc[:], Act.Tanh, bias=biasg[:, 3:4],
                                 scale=1.0)

            t2 = prd.tile([H, R], c16, tag="t2")
            nc.vector.tensor_tensor(t2[:], hT, hc[:], Alu.subtract)
            uu = prd.tile([H, R], c16, tag="uu")
            nc.vector.tensor_tensor(uu[:], rclip[:, 0:R], t2[:], Alu.mult)
            hout = outp.tile([H, R], c16, tag="hout")
            nc.vector.tensor_tensor(hout[:], uu[:], hc[:], Alu.add)

            nc.gpsimd.dma_start(ho_e[:, r0:r0 + R], hout[:])

    nc.compile()
    return nc


def _get_program(shard_rows=SHARD, zero_b=True, io_dt="bf16"):
    key = (shard_rows, zero_b, io_dt)
    if key not in _CACHE:
        _CACHE[key] = _build_program(shard_rows, zero_b, io_dt)
    return _CACHE[key]


def make_in_maps(x, h_tm1, W, U, b, x_mask, prev_has_value, has_value_tm1,
                 shard_rows=SHARD, n_cores=N_CORES, io_dt="bf16"):
    if io_dt == "bf16":
        import ml_dtypes
        cdt = ml_dtypes.bfloat16
    else:
        cdt = np.float16
    x = np.asarray(x, dtype=np.float32)
    h_tm1 = np.asarray(h_tm1, dtype=np.float32)
    W = np.asarray(W, dtype=np.float32)
    U = np.asarray(U, dtype=np.float32)
    b = np.asarray(b, dtype=np.float32)
    x_mask = np.asarray(x_mask)
    prev_has_value = np.asarray(prev_has_value)
    has_value_tm1 = np.asarray(has_value_tm1)

    w16 = np.ascontiguousarray(W, dtype=cdt)
    u16 = np.ascontiguousarray(U, dtype=cdt)
    ones1 = np.ones((1, H), cdt)
    biasg = np.zeros((H, 4), np.float32)
    biasg[:, 0] = 0.2 * b[0:D] + 0.5
    biasg[:, 1] = 0.2 * b[D:2 * D] + 0.5
    biasg[:, 2] = 0.2 * b[2 * D:3 * D] + 0.5
    biasg[:, 3] = b[3 * D:4 * D]

    jc = shard_rows // H
    in_maps = []
    for c in range(n_cores):
        sl = slice(c * shard_rows, (c + 1) * shard_rows)
        hv = has_value_tm1[sl] != 0
        merge = (x_mask[sl] * prev_has_value[sl]) != 0
        xpass = merge & ~hv           # rows whose output is x
        gru = merge & hv              # rows that really run the GRU
        # substitute x into h on x-pass rows; z-saturation then emits it
        h_eff = np.where(xpass[:, None], x[sl], h_tm1[sl])
        xt = np.ascontiguousarray(x[sl].astype(cdt).T)
        ht = np.ascontiguousarray(h_eff.astype(cdt).T)
        mbig = np.where(gru, cdt(0.0), cdt(BIGM)).reshape(1, shard_rows)
        mgc = merge.astype(np.float32).reshape(H, jc)
        hvc = hv.astype(np.float32).reshape(H, jc)
        in_maps.append({
            "xt": xt, "ht": ht, "w16": w16, "u16": u16, "ones1": ones1,
            "biasg": biasg, "mbig": mbig, "mgc": mgc, "hvc": hvc,
        })
    return in_maps


def kernel(x, h_tm1, W, U, b, x_mask, prev_has_value, has_value_tm1):
    from concourse.bass_utils import run_bass_kernel_spmd

    b = np.asarray(b, dtype=np.float32)
    zero_b = bool(np.all(b == 0.0))
    nc = _get_program(SHARD, zero_b=zero_b)
    in_maps = make_in_maps(x, h_tm1, W, U, b, x_mask, prev_has_value,
                           has_value_tm1)
    res = run_bass_kernel_spmd(nc, in_maps, list(range(N_CORES)))
    h = np.concatenate(
        [np.asarray(res.results[i]["hot"]).T.astype(np.float32)
         for i in range(N_CORES)], axis=0)
    hv = np.concatenate([res.results[i]["hvo"] for i in range(N_CORES)],
                        axis=0).astype(bool)
    return h, hv
